# revision 22
# baseline (speedup 1.0000x reference)
"""3x3 median blur (cv2 medianBlur semantics, BORDER_REPLICATE) on Trainium2.

Input:  x (64, 224, 224, 3) float32
Output: same shape; per-pixel/per-channel median of the 3x3 neighborhood.

Strategy
--------
Pure data parallel: batch 64 -> 8 cores x 8 images. Per core, rows are laid
out (w, c)-flattened: one image row is 224*3 = 672 contiguous floats; a
horizontal shift of +-1 pixel is a +-3 float shift.

Partition mapping (band-major): partition p = band*8 + img, with 16 bands
of 14 rows per image -> 128 partitions. Row chunks of S rows per Tile
iteration. The input tile is unpadded and contiguous, so each interior
iteration loads its s+2 rows (incl. re-read halo rows) with a single 3-dim
DMA and stores with one DMA; image-edge halos are clamped via split DMAs
over contiguous partition ranges in the first/last iterations. Column
replication: the vertical-stage outputs lo/hi/mid are 678-wide with their
edge columns replicated into 3-float pads by ScalarE (idle engine), which
makes every horizontal tap a uniform +-3 float shift.

Median network (exact, 18 min/max tensor ops per pixel), the classic
separable form: per-column vertical sort3 (lo/mid/hi), then horizontal
  median9 = med3( max3(lo), med3(mid), min3(hi) ).
All 18 ops run on the Vector engine (the only engine with 2-input min/max
in this compiler build); ScalarE does the edge-column pad copies.
"""

import os

import numpy as np

N_CORES = 8
B_FULL = 64
B_LOCAL = B_FULL // N_CORES  # 8 images per core
H = 224
W = 224
C = 3
WC = W * C          # 672 floats per row
WCP = WC + 2 * C    # 678 padded row
BANDS = 16          # bands per image
BAND_ROWS = H // BANDS  # 14
S = 3               # rows per tile iteration

IMG_STRIDE = H * WC       # 150528
BAND_STRIDE = BAND_ROWS * WC  # 9408

LAST_RESULT = None  # BassKernelResults of the most recent run (for test.py)


def _build_bass(repeat=1, chunks=None):
    import concourse.bacc as bacc
    import concourse.bass as bass
    import concourse.mybir as mybir
    import concourse.tile as tile

    f32 = mybir.dt.float32
    MIN = mybir.AluOpType.min
    MAX = mybir.AluOpType.max

    nc = bacc.Bacc("TRN2", target_bir_lowering=False, debug=False)

    x = nc.dram_tensor("x", [B_LOCAL, H, W, C], f32, kind="ExternalInput")
    y = nc.dram_tensor("y", [B_LOCAL, H, W, C], f32, kind="ExternalOutput")
    xt = x.ap().tensor
    yt = y.ap().tensor

    dummy = None
    if repeat != 1:
        # K-dependent input shape => distinct HLO signature per K, defeating
        # any structural compile cache that ignores the embedded BIR.
        dummy = nc.dram_tensor(f"dummyb{repeat}", [1, 128 + repeat], f32,
                               kind="ExternalInput")

    def dram_ap(t, offset, ap):
        return bass.AP(tensor=t, offset=offset, ap=ap)

    if chunks is None:
        chunks = []
        left = BAND_ROWS
        while left > 0:
            chunks.append(min(S, left))
            left -= S

    with tile.TileContext(nc) as tc:
        with (
            tc.tile_pool(name="pin", bufs=2) as pin,
            tc.tile_pool(name="pab", bufs=1) as pab,
            tc.tile_pool(name="pv", bufs=1) as pv,
            tc.tile_pool(name="ph", bufs=1) as ph,
            tc.tile_pool(name="pout", bufs=2) as pout,
        ):
            if dummy is not None:
                with tc.tile_pool(name="pdummy", bufs=1) as pd:
                    dt_ = pd.tile([1, 128 + repeat], f32, tag="dummy")
                    nc.sync.dma_start(out=dt_, in_=dummy.ap())
            for _rep in range(repeat):
                for it, s in enumerate(chunks):
                    r0 = sum(chunks[:it])
                    is_first = it == 0
                    is_last = it == len(chunks) - 1

                    # ---- load xin[128, s+2, WC] (unpadded, contiguous):
                    # band rows r0-1 .. r0+s. The (s+2)-row block is one
                    # contiguous DRAM run per (band, img), so interior
                    # iterations need a single 3-dim DMA.
                    xin = pin.tile([128, s + 2, WC], f32, tag="xin")

                    if is_first:
                        # rows 0..s valid -> tile rows 1..s+1, one run
                        nc.sync.dma_start(
                            out=xin[:, 1:s + 2, :],
                            in_=dram_ap(xt, 0,
                                        [[BAND_STRIDE, BANDS],
                                         [IMG_STRIDE, B_LOCAL],
                                         [1, (s + 1) * WC]]),
                        )
                        # top halo (tile row 0): bands>=1 read previous band's
                        # last row; band 0 clamps to image row 0.
                        nc.sync.dma_start(
                            out=xin[B_LOCAL:, 0:1, :],
                            in_=dram_ap(xt, BAND_STRIDE - WC,
                                        [[BAND_STRIDE, BANDS - 1],
                                         [IMG_STRIDE, B_LOCAL], [1, WC]]),
                        )
                        nc.sync.dma_start(
                            out=xin[:B_LOCAL, 0:1, :],
                            in_=dram_ap(xt, 0, [[IMG_STRIDE, B_LOCAL], [1, WC]]),
                        )
                    elif is_last:
                        # rows r0-1..r0+s-1 valid -> tile rows 0..s, one run
                        nc.sync.dma_start(
                            out=xin[:, 0:s + 1, :],
                            in_=dram_ap(xt, (r0 - 1) * WC,
                                        [[BAND_STRIDE, BANDS],
                                         [IMG_STRIDE, B_LOCAL],
                                         [1, (s + 1) * WC]]),
                        )
                        # bottom halo (tile row s+1): bands<=14 read next
                        # band's row 0; band 15 clamps to image row 223.
                        nc.sync.dma_start(
                            out=xin[:(BANDS - 1) * B_LOCAL, s + 1:s + 2, :],
                            in_=dram_ap(xt, BAND_STRIDE,
                                        [[BAND_STRIDE, BANDS - 1],
                                         [IMG_STRIDE, B_LOCAL], [1, WC]]),
                        )
                        nc.sync.dma_start(
                            out=xin[(BANDS - 1) * B_LOCAL:, s + 1:s + 2, :],
                            in_=dram_ap(
                                xt,
                                (BANDS - 1) * BAND_STRIDE + (BAND_ROWS - 1) * WC,
                                [[IMG_STRIDE, B_LOCAL], [1, WC]]),
                        )
                    else:
                        # interior: all s+2 rows in one DMA
                        nc.sync.dma_start(
                            out=xin[:, :, :],
                            in_=dram_ap(xt, (r0 - 1) * WC,
                                        [[BAND_STRIDE, BANDS],
                                         [IMG_STRIDE, B_LOCAL],
                                         [1, (s + 2) * WC]]),
                        )

                    # ---- vertical sort3 per column (672-wide) ----
                    xu = xin[:, 0:s, :]
                    xm = xin[:, 1:s + 1, :]
                    xd = xin[:, 2:s + 2, :]
                    av = pab.tile([128, s, WC], f32, tag="A")
                    bv = pab.tile([128, s, WC], f32, tag="B")
                    nc.vector.tensor_tensor(av, xu, xm, MIN)
                    nc.vector.tensor_tensor(bv, xu, xm, MAX)

                    # lo/hi/mid are 678-wide (pads needed by the horizontal
                    # taps); vertical results land in cols C..C+WC, ScalarE
                    # replicates the edge columns into the pads.
                    lo = pv.tile([128, s, WCP], f32, tag="lo")
                    tt = pv.tile([128, s, WC], f32, tag="t")
                    hi = pv.tile([128, s, WCP], f32, tag="hi")
                    mid = pv.tile([128, s, WCP], f32, tag="mid")
                    nc.vector.tensor_tensor(lo[:, :, C:C + WC], av, xd, MIN)
                    nc.vector.tensor_tensor(tt, bv, xd, MIN)
                    nc.vector.tensor_tensor(hi[:, :, C:C + WC], bv, xd, MAX)
                    nc.vector.tensor_tensor(mid[:, :, C:C + WC], av, tt, MAX)
                    for v in (lo, hi, mid):
                        nc.scalar.copy(out=v[:, :, 0:C], in_=v[:, :, C:2 * C])
                        nc.scalar.copy(out=v[:, :, WC + C:WC + 2 * C],
                                       in_=v[:, :, WC:WC + C])

                    # ---- horizontal; l/c/r = col offsets 0/3/6 ----
                    def L(v):
                        return v[:, :, 0:WC]

                    def Cc(v):
                        return v[:, :, C:C + WC]

                    def R(v):
                        return v[:, :, 2 * C:2 * C + WC]

                    m1 = ph.tile([128, s, WC], f32, tag="h0")
                    aa = ph.tile([128, s, WC], f32, tag="h1")
                    nc.vector.tensor_tensor(m1, L(lo), R(lo), MAX)
                    nc.vector.tensor_tensor(aa, m1, Cc(lo), MAX)

                    m2 = ph.tile([128, s, WC], f32, tag="h4")
                    cc = ph.tile([128, s, WC], f32, tag="h5")
                    nc.vector.tensor_tensor(m2, L(hi), R(hi), MIN)
                    nc.vector.tensor_tensor(cc, m2, Cc(hi), MIN)

                    t1 = ph.tile([128, s, WC], f32, tag="h0")
                    t2 = ph.tile([128, s, WC], f32, tag="h3")
                    nc.vector.tensor_tensor(t1, L(mid), Cc(mid), MIN)
                    nc.vector.tensor_tensor(t2, L(mid), Cc(mid), MAX)
                    t3 = ph.tile([128, s, WC], f32, tag="h2")
                    nc.vector.tensor_tensor(t3, t2, R(mid), MIN)
                    bb = ph.tile([128, s, WC], f32, tag="h3")
                    nc.vector.tensor_tensor(bb, t1, t3, MAX)

                    f1 = ph.tile([128, s, WC], f32, tag="h2")
                    f2 = ph.tile([128, s, WC], f32, tag="h0")
                    nc.vector.tensor_tensor(f1, aa, bb, MIN)
                    nc.vector.tensor_tensor(f2, aa, bb, MAX)
                    f3 = ph.tile([128, s, WC], f32, tag="h1")
                    nc.vector.tensor_tensor(f3, f2, cc, MIN)
                    out_t = pout.tile([128, s, WC], f32, tag="out")
                    nc.vector.tensor_tensor(out_t, f1, f3, MAX)

                    # ---- store: s rows are one contiguous run per band ----
                    nc.sync.dma_start(
                        out=dram_ap(yt, r0 * WC,
                                    [[BAND_STRIDE, BANDS],
                                     [IMG_STRIDE, B_LOCAL], [1, s * WC]]),
                        in_=out_t,
                    )

    nc.compile()
    return nc


_NC_CACHE = None


def kernel(x: np.ndarray) -> np.ndarray:
    global LAST_RESULT, _NC_CACHE
    from concourse.bass_utils import run_bass_kernel_spmd

    assert x.shape == (B_FULL, H, W, C), x.shape
    x = np.ascontiguousarray(np.asarray(x, dtype=np.float32))

    if _NC_CACHE is None:
        _NC_CACHE = _build_bass()
    nc = _NC_CACHE

    in_maps = [
        {"x": x[i * B_LOCAL:(i + 1) * B_LOCAL]} for i in range(N_CORES)
    ]
    trace = bool(int(os.environ.get("MEDIAN_TRACE", "0")))
    try:
        res = run_bass_kernel_spmd(
            nc, in_maps, core_ids=list(range(N_CORES)), trace=trace,
        )
    except ModuleNotFoundError:
        # axon NTFF profiling hook unavailable in this build
        res = run_bass_kernel_spmd(
            nc, in_maps, core_ids=list(range(N_CORES)), trace=False,
        )
    LAST_RESULT = res
    out = np.concatenate([r["y"] for r in res.results], axis=0)
    return out
